# revision 44
# baseline (speedup 1.0000x reference)
"""AttnRes pooling kernel for Trainium2 (Bass/Tile), 8-core SPMD.

Computes, for V = layer_outputs [N=12, B=4, T=2048, D=768]:
    inv_rms = rsqrt(mean(V^2, -1) + 1e-6)
    logits[n,b,t] = dot(q*w, V[n,b,t,:]) * inv_rms[n,b,t]
    alpha = softmax(logits, axis=0)   # over layer dim N
    h[b,t,d] = sum_n alpha[n,b,t] * V[n,b,t,d]

Sharding: B*T = 8192 positions split contiguously across 8 cores (1024
positions each). q*w is combined on host and replicated. Softmax is over N,
so no cross-core communication is needed.

Per-core device program (natural layout: 128 positions on partitions, D on
the free dim), software-pipelined across 8 position-tiles:
  - DVE: fused scalar_tensor_tensor (V * qw_broadcast, accum -> per-position
    dot; a couple of layers' sum-of-squares), small softmax ops, and the
    per-layer diag(alpha_n) build (identity * alpha_n, per-partition scalars).
  - ACT: activation(Square, accum_out) -> sum(V^2); Sqrt / Exp for the
    softmax; PSUM->SBUF copy of the result.
  - PE: h = sum_n diag(alpha_n) @ V_n accumulated in PSUM (12 matmuls/tile).
  - HW quirk: this walrus accepts one sync-wait per instruction, so
    _split_multiwaits hoists extras onto EventSemaphore instructions.
"""

from contextlib import ExitStack

import numpy as np

import concourse.bass as bass
import concourse.mybir as mybir
import concourse.tile as tile
from concourse import bass_utils

N_LAYERS = 12
B = 4
T = 2048
D = 768
N_CORES = 8
POS = B * T  # 8192
PPC = POS // N_CORES  # 1024 positions per core
P = 128  # SBUF partitions
NTILES = PPC // P  # 8 position-tiles per core
EPS = 1e-6

f32 = mybir.dt.float32


def _split_multiwaits(nc: bass.Bass) -> int:
    """Hoist all-but-one sync waits onto standalone InstEventSemaphore
    instructions inserted immediately before the over-subscribed instruction.

    This walrus build accepts only one sync-wait per TPB instruction, while
    bass_rust's Tile scheduler emits up to two on event-semaphore (HWDGE)
    waits. Inserting the extra waits as EventSemaphore instructions at the
    same program point on the same engine is semantically identical.
    """
    cnt = 0
    for f in nc.m.functions:
        for bb in f.blocks:
            insts = bb.instructions
            i = 0
            while i < len(insts):
                inst = insts[i]
                si = inst.sync_info
                if si is not None and si.on_wait is not None and len(si.on_wait) > 1:
                    waits = list(si.on_wait)
                    for j, w in enumerate(waits[:-1]):
                        ev = mybir.InstEventSemaphore(
                            name=f"{inst.name}-wsplit{j}",
                            engine=inst.engine,
                            sync_info=mybir.SyncInfo(on_wait=[w], on_update=[]),
                        )
                        insts.insert(i, ev)
                        i += 1
                        cnt += 1
                    inst.sync_info = mybir.SyncInfo(
                        on_wait=[waits[-1]], on_update=list(si.on_update or [])
                    )
                i += 1
    return cnt


def _build_bass(
    reps: int = 1,
    do_dot: bool = True,
    do_sq: bool = True,
    do_combine: bool = True,
    vbufs: int = 4,
    sbufs: int = 2,
    dbufs: int = 4,
    pbufs: int = 2,
    skew: int = 8,
    mode: str = "fp32",  # fp32 | gpscopy | dmacast
    bbufs: int = 3,
    hcopy_dve: bool = False,
    dve_sq: int = 2,  # how many layers' sum-of-squares go to DVE instead of ACT
    loop_reps: int = 1,  # hardware For_i loop around the whole program (timing)
    big_dma: bool = False,  # dmacast: one casting DMA per tile instead of 12
    pe_f32: int = 0,  # gpscopy: layers whose combine matmul reads fp32 V directly
    diag_gps: bool = False,  # build diag tiles on GPSIMD instead of DVE
) -> bass.Bass:
    nc = bass.Bass("TRN2")
    Alu = mybir.AluOpType
    Act = mybir.ActivationFunctionType
    combine_bf16 = mode in ("gpscopy", "dmacast")
    idt = mybir.dt.bfloat16 if combine_bf16 else f32

    qdt = mybir.dt.bfloat16 if mode == "dmacast" else f32
    lo = nc.dram_tensor("lo", [N_LAYERS, PPC, D], f32, kind="ExternalInput").ap()
    qwb = nc.dram_tensor("qwb", [P, D], qdt, kind="ExternalInput").ap()
    ident = nc.dram_tensor("ident", [P, P], idt, kind="ExternalInput").ap()
    out = nc.dram_tensor("out", [PPC, D], f32, kind="ExternalOutput").ap()

    with ExitStack() as ctx:
        tc = ctx.enter_context(tile.TileContext(nc))
        singles = ctx.enter_context(tc.tile_pool(name="singles", bufs=1))
        vpool = ctx.enter_context(tc.tile_pool(name="v", bufs=vbufs))
        spool = ctx.enter_context(tc.tile_pool(name="small", bufs=sbufs))
        dpool = ctx.enter_context(tc.tile_pool(name="diag", bufs=dbufs))
        ppool = ctx.enter_context(tc.tile_pool(name="psum", bufs=pbufs, space="PSUM"))

        bf16 = mybir.dt.bfloat16
        cdt = bf16 if combine_bf16 else f32
        bpool = (
            ctx.enter_context(tc.tile_pool(name="vb", bufs=bbufs))
            if combine_bf16
            else None
        )

        qwb_t = singles.tile([P, D], qdt)
        nc.sync.dma_start(out=qwb_t, in_=qwb)
        ident_t = singles.tile([P, P], cdt)
        nc.sync.dma_start(out=ident_t, in_=ident)
        ident_f32 = nc.dram_tensor("ident_f32", [P, P], f32, kind="ExternalInput").ap()
        ident_f32_t = singles.tile([P, P], f32)
        nc.sync.dma_start(out=ident_f32_t, in_=ident_f32)
        eps_t = singles.tile([P, 1], f32)
        nc.vector.memset(eps_t, EPS)
        dummy_v = singles.tile([P, 1], f32)
        dummy_a = singles.tile([P, 1], f32)

        f32r = mybir.dt.float32r
        ncomb = N_LAYERS if do_combine else 1

        def loads(i):
            """Issue tile i's loads; return (combine-tensors, reduce-tensors,
            dots, s2)."""
            dots = spool.tile([P, N_LAYERS], f32, tag="dots")
            s2 = spool.tile([P, N_LAYERS], f32, tag="s2")
            if mode == "dmacast":
                vb = bpool.tile([P, N_LAYERS, D], bf16, tag="vb")
                cts = [vb[:, n, :] for n in range(N_LAYERS)]
                if big_dma:
                    # one casting DMA for all 12 layers: iterate the HBM side
                    # in (pos, n, d) order to match the SBUF tile layout;
                    # contiguous runs stay 768 elements.
                    src = lo[:, i * P : (i + 1) * P, :].rearrange("n p d -> p n d")
                    nc.gpsimd.dma_start(out=vb, in_=src)
                else:
                    for n in range(N_LAYERS):
                        nc.gpsimd.dma_start(
                            out=cts[n], in_=lo[n, i * P : (i + 1) * P, :]
                        )
                rts = cts
            else:
                v = vpool.tile([P, N_LAYERS, D], f32, tag="v")
                vts = [v[:, n, :] for n in range(N_LAYERS)]
                for n in range(N_LAYERS):
                    nc.sync.dma_start(out=vts[n], in_=lo[n, i * P : (i + 1) * P, :])
                if mode == "gpscopy":
                    vb = bpool.tile([P, N_LAYERS, D], bf16, tag="vb")
                    cts = [vb[:, n, :] for n in range(N_LAYERS)]
                else:
                    cts = vts
                rts = vts
            return cts, rts, dots, s2

        def reduces(state, n0, n1):
            """Per-layer reductions for layers [n0, n1): dot on DVE,
            sum-of-squares on ACT (first dve_sq layers on DVE)."""
            cts, rts, dots, s2 = state
            for n in range(n0, n1):
                if do_dot:
                    nc.vector.scalar_tensor_tensor(
                        out=dummy_v.broadcast_to((P, D)),
                        in0=rts[n],
                        scalar=1.0,
                        in1=qwb_t,
                        op0=Alu.mult,
                        op1=Alu.mult,
                        accum_out=dots[:, n : n + 1],
                    )
                else:
                    nc.vector.memset(dots[:, n : n + 1], 0.1)
                if do_sq:
                    if n < dve_sq:
                        # sum of squares on DVE (one fused pass)
                        nc.vector.scalar_tensor_tensor(
                            out=dummy_v.broadcast_to((P, D)),
                            in0=rts[n],
                            scalar=1.0,
                            in1=rts[n],
                            op0=Alu.mult,
                            op1=Alu.mult,
                            accum_out=s2[:, n : n + 1],
                        )
                    else:
                        nc.scalar.activation(
                            out=dummy_a.broadcast_to((P, D)),
                            in_=rts[n],
                            func=Act.Square,
                            accum_out=s2[:, n : n + 1],
                        )
                else:
                    nc.vector.memset(s2[:, n : n + 1], 1.0)
                if mode == "gpscopy" and n >= pe_f32:
                    nc.gpsimd.tensor_copy(out=cts[n], in_=rts[n])

        def tail(i, state):
            """Softmax over layers, then h = sum_n alpha_n V_n on PE via
            accumulated diag(alpha_n) @ V_n, then store."""
            vts, _, dots, s2 = state
            rms = spool.tile([P, N_LAYERS], f32, tag="rms")
            nc.scalar.activation(
                out=rms, in_=s2, func=Act.Sqrt, scale=1.0 / D, bias=eps_t
            )
            invr = spool.tile([P, N_LAYERS], f32, tag="invr")
            nc.vector.reciprocal(invr, rms)
            logits = spool.tile([P, N_LAYERS], f32, tag="logits")
            nc.vector.tensor_mul(logits, dots, invr)
            negm = spool.tile([P, 1], f32, tag="negm")
            nc.vector.tensor_reduce(
                negm, logits, axis=mybir.AxisListType.X, op=Alu.max, negate=True
            )
            e = spool.tile([P, N_LAYERS], f32, tag="e")
            se = spool.tile([P, 1], f32, tag="se")
            nc.scalar.activation(
                out=e, in_=logits, func=Act.Exp, bias=negm, scale=1.0, accum_out=se
            )
            ise = spool.tile([P, 1], f32, tag="ise")
            nc.vector.reciprocal(ise, se)

            # build all diag(alpha_n) tiles first so the PE matmuls run
            # back-to-back (keeps the PE p-state ramp warm).
            h = ppool.tile([P, D], f32)
            diags = dpool.tile([P, N_LAYERS, P], cdt)
            diag_eng = nc.gpsimd if diag_gps else nc.vector
            for n in range(ncomb):
                diag_eng.tensor_scalar(
                    out=diags[:, n, :],
                    in0=ident_t,
                    scalar1=e[:, n : n + 1],
                    scalar2=ise,
                    op0=Alu.mult,
                    op1=Alu.mult,
                )
            if mode == "gpscopy" and pe_f32 > 0:
                # PE reads fp32 V directly for the first pe_f32 layers (PE has
                # slack; saves GPSIMD copies). fp32 matmuls need an fp32 diag.
                fdiags = dpool.tile([P, max(pe_f32, 1), P], f32, tag="fdiags")
                for n in range(pe_f32):
                    diag_eng.tensor_scalar(
                        out=fdiags[:, n, :],
                        in0=ident_f32_t,
                        scalar1=e[:, n : n + 1],
                        scalar2=ise,
                        op0=Alu.mult,
                        op1=Alu.mult,
                    )
            _, rts_t, _, _ = state
            for n in range(ncomb):
                use_f32 = mode == "gpscopy" and n < pe_f32
                lhsT_n = fdiags[:, n, :] if use_f32 else diags[:, n, :]
                rhs_src = rts_t[n] if use_f32 else vts[n]
                for c0 in range(0, D, 512):
                    c1 = min(c0 + 512, D)
                    nc.tensor.matmul(
                        out=h[:, c0:c1],
                        lhsT=lhsT_n,
                        rhs=rhs_src[:, c0:c1],
                        start=(n == 0),
                        stop=(n == ncomb - 1),
                    )
            h_sb = spool.tile([P, D], f32, tag="h_sb")
            if hcopy_dve:
                nc.vector.tensor_copy(h_sb, h)
            else:
                nc.scalar.copy(h_sb, h)
            nc.sync.dma_start(out=out[i * P : (i + 1) * P, :], in_=h_sb)

        # software pipeline: optionally emit tile i's bulk before tile i-1's
        # tail so the softmax ping-pong hides behind the next tile's
        # streaming work (skew=1); skew=0 is the straight order.
        def body():
            # skew = number of next-tile reduce-layers emitted before the
            # previous tile's tail (0 = straight order, 12 = full bulk).
            tiles = [t for _ in range(reps) for t in range(NTILES)]
            pending = None
            for i in tiles:
                state = loads(i)
                reduces(state, 0, skew)
                if pending is not None:
                    tail(*pending)
                reduces(state, skew, N_LAYERS)
                pending = (i, state)
            tail(*pending)

        if loop_reps > 1:
            with tc.For_i(0, loop_reps, 1):
                body()
        else:
            body()

    _split_multiwaits(nc)
    return nc


def _make_in_maps(layer_outputs, pseudo_query, key_norm_weight, mode="fp32"):
    V = np.ascontiguousarray(np.asarray(layer_outputs, dtype=np.float32)).reshape(
        N_LAYERS, POS, D
    )
    qw = np.asarray(pseudo_query, dtype=np.float32) * np.asarray(
        key_norm_weight, dtype=np.float32
    )
    import ml_dtypes

    qwb = np.ascontiguousarray(np.broadcast_to(qw[None, :], (P, D))).astype(
        ml_dtypes.bfloat16 if mode == "dmacast" else np.float32
    )
    if mode in ("gpscopy", "dmacast"):
        ident = np.eye(P, dtype=ml_dtypes.bfloat16)
    else:
        ident = np.eye(P, dtype=np.float32)
    ident_f32 = np.eye(P, dtype=np.float32)
    in_maps = []
    for c in range(N_CORES):
        shard = np.ascontiguousarray(V[:, c * PPC : (c + 1) * PPC, :])
        in_maps.append(
            {"lo": shard, "qwb": qwb, "ident": ident, "ident_f32": ident_f32}
        )
    return in_maps


MODE = "fp32"

# tuned per-mode build configs (TimelineSim-guided, HW-validated)
MODE_CFG = {
    "fp32": dict(skew=8, dve_sq=2, vbufs=4),
    "gpscopy": dict(skew=12, dve_sq=3, vbufs=3, bbufs=4),
}


def kernel(layer_outputs, pseudo_query, key_norm_weight):
    nc = _build_bass(mode=MODE, **MODE_CFG[MODE])
    in_maps = _make_in_maps(layer_outputs, pseudo_query, key_norm_weight, mode=MODE)
    res = bass_utils.run_bass_kernel_spmd(nc, in_maps, core_ids=list(range(N_CORES)))
    outs = [r["out"] for r in res.results]
    return np.concatenate(outs, axis=0).reshape(B, T, D).astype(np.float32)
